# revision 7
# baseline (speedup 1.0000x reference)
"""AutoCorrelation (factor=3) Trainium2 kernel, 8-core batch-parallel.

Math: the reference's corr tensor [B,L,2047] is only ever used through its
mean over L. That mean collapses to quadratic forms of the Gram matrix
M_b = q_b^T k_b (one [512,512] matmul per batch):
    Zbar[f] = c_f^T M c_f + s_f^T M s_f  + i (c_f^T M s_f - s_f^T M c_f)
mean_value = irfft(Zbar/L, 2047) is a tiny [513]->[2047] cos/sin matmul
(done on host), and the final roll-sum is a circular correlation done
spectrally with constant DFT matrices (all dense matmuls on device).

The f=512 Nyquist column is packed into the (always-zero) sin slot f=0 on
both device stages so every tensor is exactly 512 wide (4x128 tiles, all
even sizes -> fp32r-legal). NEFF1's U-stage uses four independent host
tables so the packed slots unpack cleanly:
    u1 = uc1*T1 + us1*T2   (col0: 1*T1[:,0])         -> Zr[0..511]
    u2 = uc2*T2 - us1*T1   (col0: c512*T2[:,0])      -> [Zr[512], Zi[1..511]]
where the T-stage sin table has col0 := cos(pi*d) so T2[:,0] = M c512.

NEFF1 (per core, batch b): N = k^T q; T1 = N^T tcT; T2 = N^T tsT;
    u1/u2 elementwise; Zr/Zi = partition-sum (gpsimd) -> zout [2,512]
Host: mean_value -> top-20 -> softmax weights w[b], batch-0 shifts ->
    per-batch spectral coefficient vectors a,b,c,d [512].
NEFF2 (per core): Vr/Vi = DFT_L(v) (Nyquist packed into Vi row 0);
    Hr = a*Vr + b*Vi; Hi = c*Vi + d*Vr; out = ICr^T Hr + ICs^T Hi.
"""
import math
import numpy as np

from contextlib import ExitStack
from concourse import bass, mybir, tile, bacc
from concourse.bass_utils import run_bass_kernel_spmd

B, L, D = 8, 1024, 512
NF = L // 2 + 1      # 513
T = 2 * L - 1        # 2047
K = int(3 * math.log(float(L)))  # 20
F32 = mybir.dt.float32

# matmul compute dtype: float32 (safe) or float32r (full-rate fp32 path)
MM_DT = mybir.dt.float32r

NCORES = 8
CORE_IDS = list(range(NCORES))

_cache = {}


# ---------------------------------------------------------------- tables
def _tables():
    if 'tables' in _cache:
        return _cache['tables']
    d = np.arange(D)
    l_idx = np.arange(L)
    p = np.arange(512)
    f = np.arange(NF)

    ang1 = 2 * np.pi * np.outer(d, p) / L              # [512, 512]
    cosd = np.cos(ang1)
    sind = np.sin(ang1)
    c512 = np.cos(np.pi * d)                           # (-1)^d

    tct = cosd.copy()                                  # col0 = 1
    tst = sind.copy(); tst[:, 0] = c512                # T-stage pack
    uc1 = cosd.copy()
    uc2 = cosd.copy(); uc2[:, 0] = c512
    us1 = sind.copy()                                  # col0 = 0

    ang2 = 2 * np.pi * np.outer(f, np.arange(T)) / T   # [513, 2047]
    alpha = np.full(NF, 2.0); alpha[0] = 1.0
    C2 = alpha[:, None] * np.cos(ang2) / (T * L)
    S2 = -2.0 * np.sin(ang2) / (T * L); S2[0] = 0.0

    ang = 2 * np.pi * np.outer(l_idx, p) / L           # [1024, 512]
    fc = np.cos(ang)
    fs = -np.sin(ang); fs[:, 0] = (-1.0) ** l_idx      # col0 := Nyquist row
    alp = np.full(512, 2.0); alp[0] = 1.0
    icr = (alp[:, None] * np.cos(ang.T)) / L           # [512, 1024]
    ics = (-2.0 * np.sin(ang.T)) / L
    ics[0, :] = ((-1.0) ** l_idx) / L

    tabs = dict(
        tct=tct.astype(np.float32), tst=tst.astype(np.float32),
        uc1=uc1.astype(np.float32), uc2=uc2.astype(np.float32),
        us1=us1.astype(np.float32),
        C2=C2, S2=S2,
        fc=fc.astype(np.float32), fs=fs.astype(np.float32),
        icr=icr.astype(np.float32), ics=ics.astype(np.float32),
    )
    _cache['tables'] = tabs
    return tabs


# ---------------------------------------------------------------- NEFF 1
def build_neff1():
    nc = bacc.Bacc(None, target_bir_lowering=False, debug=False)
    q_d = nc.declare_dram_parameter('q', [L, D], MM_DT, isOutput=False)
    k_d = nc.declare_dram_parameter('k', [L, D], MM_DT, isOutput=False)
    tct_d = nc.declare_dram_parameter('tct', [D, 512], MM_DT, isOutput=False)
    tst_d = nc.declare_dram_parameter('tst', [D, 512], MM_DT, isOutput=False)
    uc1_d = nc.declare_dram_parameter('uc1', [D, 512], F32, isOutput=False)
    uc2_d = nc.declare_dram_parameter('uc2', [D, 512], F32, isOutput=False)
    us1_d = nc.declare_dram_parameter('us1', [D, 512], F32, isOutput=False)
    z_d = nc.declare_dram_parameter('zout', [2, 512], F32, isOutput=True)

    LT, DT = L // 128, D // 128        # 8, 4

    with tile.TileContext(nc) as tc, ExitStack() as ctx:
        pool = ctx.enter_context(tc.tile_pool(name='sb', bufs=1))
        psum = ctx.enter_context(
            tc.tile_pool(name='ps', bufs=2, space=bass.MemorySpace.PSUM))

        q_sb = pool.tile([128, LT, D], MM_DT)
        k_sb = pool.tile([128, LT, D], MM_DT)
        tct_sb = pool.tile([128, DT, 512], MM_DT)
        tst_sb = pool.tile([128, DT, 512], MM_DT)
        uc1_sb = pool.tile([128, DT, 512], F32)
        uc2_sb = pool.tile([128, DT, 512], F32)
        us1_sb = pool.tile([128, DT, 512], F32)
        for i in range(LT):
            nc.sync.dma_start(q_sb[:, i, :], q_d[i * 128:(i + 1) * 128, :])
            nc.sync.dma_start(k_sb[:, i, :], k_d[i * 128:(i + 1) * 128, :])
        for i in range(DT):
            nc.sync.dma_start(tct_sb[:, i, :], tct_d[i * 128:(i + 1) * 128, :])
            nc.sync.dma_start(tst_sb[:, i, :], tst_d[i * 128:(i + 1) * 128, :])
            nc.sync.dma_start(uc1_sb[:, i, :], uc1_d[i * 128:(i + 1) * 128, :])
            nc.sync.dma_start(uc2_sb[:, i, :], uc2_d[i * 128:(i + 1) * 128, :])
            nc.sync.dma_start(us1_sb[:, i, :], us1_d[i * 128:(i + 1) * 128, :])

        # N[d2, d1] = sum_l k[l,d2] q[l,d1]
        n_sb = pool.tile([128, DT, D], MM_DT)
        for t2 in range(DT):
            pn = psum.tile([128, D], F32)
            for lt in range(LT):
                nc.tensor.matmul(
                    pn[:],
                    k_sb[:, lt, t2 * 128:(t2 + 1) * 128],
                    q_sb[:, lt, :],
                    start=(lt == 0), stop=(lt == LT - 1))
            nc.vector.tensor_copy(n_sb[:, t2, :], pn[:])

        # T1[d1, p] = sum_d2 N[d2,d1] tct[d2,p];  T2 with tst
        t1_sb = pool.tile([128, DT, 512], F32)
        t2_sb = pool.tile([128, DT, 512], F32)
        for d1t in range(DT):
            pt = psum.tile([128, 512], F32, tag='pt')
            for t2 in range(DT):
                nc.tensor.matmul(
                    pt[:],
                    n_sb[:, t2, d1t * 128:(d1t + 1) * 128],
                    tct_sb[:, t2, :],
                    start=(t2 == 0), stop=(t2 == DT - 1))
            nc.vector.tensor_copy(t1_sb[:, d1t, :], pt[:])
            pt2 = psum.tile([128, 512], F32, tag='pt')
            for t2 in range(DT):
                nc.tensor.matmul(
                    pt2[:],
                    n_sb[:, t2, d1t * 128:(d1t + 1) * 128],
                    tst_sb[:, t2, :],
                    start=(t2 == 0), stop=(t2 == DT - 1))
            nc.vector.tensor_copy(t2_sb[:, d1t, :], pt2[:])

        # u1 = uc1*T1 + us1*T2 ; u2 = uc2*T2 - us1*T1
        u1_sb = pool.tile([128, DT, 512], F32)
        u2_sb = pool.tile([128, DT, 512], F32)
        for t in range(DT):
            tmp = pool.tile([128, 512], F32, tag='tmp')
            nc.vector.tensor_mul(tmp[:], us1_sb[:, t, :], t2_sb[:, t, :])
            nc.vector.tensor_mul(u1_sb[:, t, :], uc1_sb[:, t, :], t1_sb[:, t, :])
            nc.vector.tensor_add(u1_sb[:, t, :], u1_sb[:, t, :], tmp[:])
            tmp2 = pool.tile([128, 512], F32, tag='tmp2')
            nc.vector.tensor_mul(tmp2[:], us1_sb[:, t, :], t1_sb[:, t, :])
            nc.vector.tensor_mul(u2_sb[:, t, :], uc2_sb[:, t, :], t2_sb[:, t, :])
            nc.vector.tensor_sub(u2_sb[:, t, :], u2_sb[:, t, :], tmp2[:])

        # Zr = sum_d u1, Zi = sum_d u2: two-level partition reduce
        # (vector adds across the 4 d-tiles, then gpsimd C-axis reduce)
        z_sb = pool.tile([1, 2, 512], F32)
        for row, u_sb in ((0, u1_sb), (1, u2_sb)):
            acc = pool.tile([128, 512], F32, tag='acc')
            nc.vector.tensor_add(acc[:], u_sb[:, 0, :], u_sb[:, 1, :])
            acc2 = pool.tile([128, 512], F32, tag='acc2')
            nc.vector.tensor_add(acc2[:], u_sb[:, 2, :], u_sb[:, 3, :])
            nc.vector.tensor_add(acc[:], acc[:], acc2[:])
            nc.gpsimd.tensor_reduce(
                z_sb[:, row, :], acc[:],
                axis=mybir.AxisListType.C, op=mybir.AluOpType.add)
        nc.sync.dma_start(z_d[0:1, :], z_sb[:, 0, :])
        nc.sync.dma_start(z_d[1:2, :], z_sb[:, 1, :])

    nc.finalize()
    return nc


# ---------------------------------------------------------------- NEFF 2
def build_neff2():
    nc = bacc.Bacc(None, target_bir_lowering=False, debug=False)
    v_d = nc.declare_dram_parameter('v', [L, D], MM_DT, isOutput=False)
    fc_d = nc.declare_dram_parameter('fc', [L, 512], MM_DT, isOutput=False)
    fs_d = nc.declare_dram_parameter('fs', [L, 512], MM_DT, isOutput=False)
    icr_d = nc.declare_dram_parameter('icr', [512, L], MM_DT, isOutput=False)
    ics_d = nc.declare_dram_parameter('ics', [512, L], MM_DT, isOutput=False)
    a_d = nc.declare_dram_parameter('va', [512, 1], F32, isOutput=False)
    b_d = nc.declare_dram_parameter('vb', [512, 1], F32, isOutput=False)
    c_d = nc.declare_dram_parameter('vc', [512, 1], F32, isOutput=False)
    d_d = nc.declare_dram_parameter('vd', [512, 1], F32, isOutput=False)
    o_d = nc.declare_dram_parameter('out', [L, D], F32, isOutput=True)

    LT, PT = L // 128, 512 // 128      # 8, 4

    with tile.TileContext(nc) as tc, ExitStack() as ctx:
        pool = ctx.enter_context(tc.tile_pool(name='sb', bufs=1))
        outp = ctx.enter_context(tc.tile_pool(name='op', bufs=3))
        psum = ctx.enter_context(
            tc.tile_pool(name='ps', bufs=2, space=bass.MemorySpace.PSUM))
        psum_o = ctx.enter_context(
            tc.tile_pool(name='pso', bufs=2, space=bass.MemorySpace.PSUM))

        v_sb = pool.tile([128, LT, D], MM_DT)
        fc_sb = pool.tile([128, LT, 512], MM_DT)
        fs_sb = pool.tile([128, LT, 512], MM_DT)
        icr_sb = pool.tile([128, PT, L], MM_DT)
        ics_sb = pool.tile([128, PT, L], MM_DT)
        a_sb = pool.tile([128, PT, 1], F32)
        b_sb = pool.tile([128, PT, 1], F32)
        c_sb = pool.tile([128, PT, 1], F32)
        d_sb = pool.tile([128, PT, 1], F32)
        for i in range(LT):
            nc.sync.dma_start(v_sb[:, i, :], v_d[i * 128:(i + 1) * 128, :])
            nc.sync.dma_start(fc_sb[:, i, :], fc_d[i * 128:(i + 1) * 128, :])
            nc.sync.dma_start(fs_sb[:, i, :], fs_d[i * 128:(i + 1) * 128, :])
        for i in range(PT):
            nc.sync.dma_start(icr_sb[:, i, :], icr_d[i * 128:(i + 1) * 128, :])
            nc.sync.dma_start(ics_sb[:, i, :], ics_d[i * 128:(i + 1) * 128, :])
            nc.sync.dma_start(a_sb[:, i, :], a_d[i * 128:(i + 1) * 128, :])
            nc.sync.dma_start(b_sb[:, i, :], b_d[i * 128:(i + 1) * 128, :])
            nc.sync.dma_start(c_sb[:, i, :], c_d[i * 128:(i + 1) * 128, :])
            nc.sync.dma_start(d_sb[:, i, :], d_d[i * 128:(i + 1) * 128, :])

        # forward DFT along l: Vr[p,d], Vi[p,d]
        hr_sb = pool.tile([128, PT, D], MM_DT)
        hi_sb = pool.tile([128, PT, D], MM_DT)
        for pt in range(PT):
            pvr = psum.tile([128, D], F32, tag='pv')
            for lt in range(LT):
                nc.tensor.matmul(
                    pvr[:],
                    fc_sb[:, lt, pt * 128:(pt + 1) * 128],
                    v_sb[:, lt, :],
                    start=(lt == 0), stop=(lt == LT - 1))
            pvi = psum.tile([128, D], F32, tag='pv')
            for lt in range(LT):
                nc.tensor.matmul(
                    pvi[:],
                    fs_sb[:, lt, pt * 128:(pt + 1) * 128],
                    v_sb[:, lt, :],
                    start=(lt == 0), stop=(lt == LT - 1))
            # Hr = a*Vr + b*Vi ; Hi = c*Vi + d*Vr  (scalar = per-partition)
            tmp = pool.tile([128, D], F32, tag='htmp')
            nc.vector.tensor_scalar_mul(tmp[:], pvi[:], b_sb[:, pt, :])
            nc.vector.scalar_tensor_tensor(
                hr_sb[:, pt, :], pvr[:], a_sb[:, pt, :], tmp[:],
                mybir.AluOpType.mult, mybir.AluOpType.add)
            tmp2 = pool.tile([128, D], F32, tag='htmp2')
            nc.vector.tensor_scalar_mul(tmp2[:], pvr[:], d_sb[:, pt, :])
            nc.vector.scalar_tensor_tensor(
                hi_sb[:, pt, :], pvi[:], c_sb[:, pt, :], tmp2[:],
                mybir.AluOpType.mult, mybir.AluOpType.add)

        # inverse DFT: out[l,d] = sum_p icr[p,l] Hr[p,d] + ics[p,l] Hi[p,d]
        for lt in range(LT):
            po = psum_o.tile([128, D], F32)
            for pt in range(PT):
                nc.tensor.matmul(
                    po[:],
                    icr_sb[:, pt, lt * 128:(lt + 1) * 128],
                    hr_sb[:, pt, :],
                    start=(pt == 0), stop=False)
                nc.tensor.matmul(
                    po[:],
                    ics_sb[:, pt, lt * 128:(lt + 1) * 128],
                    hi_sb[:, pt, :],
                    start=False, stop=(pt == PT - 1))
            o_sb = outp.tile([128, D], F32)
            nc.vector.tensor_copy(o_sb[:], po[:])
            nc.sync.dma_start(o_d[lt * 128:(lt + 1) * 128, :], o_sb[:])

    nc.finalize()
    return nc


# ---------------------------------------------------------------- driver
def _get_graphs():
    if 'nc1' not in _cache:
        _cache['nc1'] = build_neff1()
        _cache['nc2'] = build_neff2()
    return _cache['nc1'], _cache['nc2']


def kernel(queries, keys, values, _trace=False):
    tabs = _tables()
    nc1, nc2 = _get_graphs()
    q = np.ascontiguousarray(np.asarray(queries, np.float32))
    k = np.ascontiguousarray(np.asarray(keys, np.float32))
    v = np.ascontiguousarray(np.asarray(values, np.float32))

    in1 = [{'q': q[b], 'k': k[b],
            'tct': tabs['tct'], 'tst': tabs['tst'],
            'uc1': tabs['uc1'], 'uc2': tabs['uc2'], 'us1': tabs['us1']}
           for b in range(B)]
    r1 = run_bass_kernel_spmd(nc1, in1, core_ids=CORE_IDS, trace=_trace)
    z = np.stack([r1.results[b]['zout'] for b in range(B)])   # [B, 2, 512]

    # unpack: Zr[0..511] = z[:,0,:]; Zr[512] = z[:,1,0]; Zi[1..511] = z[:,1,1:]
    Zr = np.concatenate([z[:, 0, :], z[:, 1, 0:1]], axis=1)   # [B, 513]
    Zi = np.concatenate(
        [np.zeros((B, 1)), z[:, 1, 1:], np.zeros((B, 1))], axis=1)
    mean_value = Zr @ tabs['C2'] + Zi @ tabs['S2']            # [B, T]
    ind = np.argsort(-mean_value, axis=-1, kind='stable')[:, :K]
    val = np.take_along_axis(mean_value, ind, axis=-1)
    e = np.exp(val - val.max(-1, keepdims=True))
    w = e / e.sum(-1, keepdims=True)                          # [B, K]
    shifts = ind[0]                                           # [K]

    f = np.arange(NF)
    ang = 2 * np.pi * np.outer(f, shifts) / L                 # [513, K]
    cosm, sinm = np.cos(ang), np.sin(ang)
    cr = w @ cosm.T                                           # [B, 513]
    ci = -(w @ sinm.T)
    a_v = cr[:, :512].copy()
    b_v = ci[:, :512].copy(); b_v[:, 0] = 0.0
    c_v = cr[:, :512].copy(); c_v[:, 0] = cr[:, 512]
    d_v = -ci[:, :512].copy(); d_v[:, 0] = 0.0

    in2 = [{'v': v[b],
            'fc': tabs['fc'], 'fs': tabs['fs'],
            'icr': tabs['icr'], 'ics': tabs['ics'],
            'va': np.ascontiguousarray(a_v[b].reshape(512, 1), np.float32),
            'vb': np.ascontiguousarray(b_v[b].reshape(512, 1), np.float32),
            'vc': np.ascontiguousarray(c_v[b].reshape(512, 1), np.float32),
            'vd': np.ascontiguousarray(d_v[b].reshape(512, 1), np.float32)}
           for b in range(B)]
    r2 = run_bass_kernel_spmd(nc2, in2, core_ids=CORE_IDS, trace=_trace)
    out = np.stack([r2.results[b]['out'] for b in range(B)])  # [B, L, D]

    kernel._last_exec_ns = (
        (r1.exec_time_ns or 0) + (r2.exec_time_ns or 0)
        if (r1.exec_time_ns or r2.exec_time_ns) else None)
    kernel._last_results = (r1, r2)
    return out.astype(np.float32)


# revision 13
# speedup vs baseline: 1.9852x; 1.9852x over previous
"""AutoCorrelation (factor=3) Trainium2 kernel, 8-core batch-parallel.

Math: the reference's corr tensor [B,L,2047] is only ever used through its
mean over L. That mean collapses to quadratic forms of the Gram matrix
M_b = q_b^T k_b (one [512,512] matmul per batch):
    Zbar[f] = c_f^T M c_f + s_f^T M s_f  + i (c_f^T M s_f - s_f^T M c_f)
mean_value = irfft(Zbar/L, 2047) is a tiny [513]->[2047] cos/sin matmul
(done on host), and the final roll-sum is a circular correlation done
spectrally with constant DFT matrices (all dense matmuls on device).

The f=512 Nyquist column is packed into the (always-zero) sin slot f=0 on
both device stages so every tensor is exactly 512 wide (4x128 tiles, all
even sizes -> fp32r-legal). NEFF1's U-stage uses four independent host
tables so the packed slots unpack cleanly:
    u1 = uc1*T1 + us1*T2   (col0: 1*T1[:,0])         -> Zr[0..511]
    u2 = uc2*T2 - us1*T1   (col0: c512*T2[:,0])      -> [Zr[512], Zi[1..511]]
where the T-stage sin table has col0 := cos(pi*d) so T2[:,0] = M c512.

NEFF1 (per core, batch b): N = k^T q; T1 = N^T tcT; T2 = N^T tsT;
    u1/u2 elementwise; Zr/Zi = partition-sum (gpsimd) -> zout [2,512]
Host: mean_value -> top-20 -> softmax weights w[b], batch-0 shifts ->
    per-batch spectral coefficient vectors a,b,c,d [512].
NEFF2 (per core): Vr/Vi = DFT_L(v) (Nyquist packed into Vi row 0);
    Hr = a*Vr + b*Vi; Hi = c*Vi + d*Vr; out = ICr^T Hr + ICs^T Hi.
"""
import math
import numpy as np

from contextlib import ExitStack
from concourse import bass, mybir, tile, bacc
from concourse.bass_utils import run_bass_kernel_spmd

B, L, D = 8, 1024, 512
NF = L // 2 + 1      # 513
T = 2 * L - 1        # 2047
K = int(3 * math.log(float(L)))  # 20
F32 = mybir.dt.float32

# matmul compute dtype: float32 (safe) or float32r (full-rate fp32 path)
MM_DT = mybir.dt.float32r

NCORES = 8
CORE_IDS = list(range(NCORES))

_cache = {}


# ---------------------------------------------------------------- tables
def _tables():
    if 'tables' in _cache:
        return _cache['tables']
    d = np.arange(D)
    l_idx = np.arange(L)
    p = np.arange(512)
    f = np.arange(NF)

    ang1 = 2 * np.pi * np.outer(d, p) / L              # [512, 512]
    cosd = np.cos(ang1)
    sind = np.sin(ang1)
    c512 = np.cos(np.pi * d)                           # (-1)^d

    tct = cosd.copy()                                  # col0 = 1
    tst = sind.copy(); tst[:, 0] = c512                # T-stage pack
    uc1 = cosd.copy()
    uc2 = cosd.copy(); uc2[:, 0] = c512
    us1 = sind.copy()                                  # col0 = 0

    ang2 = 2 * np.pi * np.outer(f, np.arange(T)) / T   # [513, 2047]
    alpha = np.full(NF, 2.0); alpha[0] = 1.0
    C2 = alpha[:, None] * np.cos(ang2) / (T * L)
    S2 = -2.0 * np.sin(ang2) / (T * L); S2[0] = 0.0

    ang = 2 * np.pi * np.outer(l_idx, p) / L           # [1024, 512]
    fc = np.cos(ang)
    fs = -np.sin(ang); fs[:, 0] = (-1.0) ** l_idx      # col0 := Nyquist row
    alp = np.full(512, 2.0); alp[0] = 1.0
    icr = (alp[:, None] * np.cos(ang.T)) / L           # [512, 1024]
    ics = (-2.0 * np.sin(ang.T)) / L
    ics[0, :] = ((-1.0) ** l_idx) / L

    tabs = dict(
        tct=tct.astype(np.float32), tst=tst.astype(np.float32),
        uc1=uc1.astype(np.float32), uc2=uc2.astype(np.float32),
        us1=us1.astype(np.float32),
        C2=C2, S2=S2,
        fc=fc.astype(np.float32), fs=fs.astype(np.float32),
        icr=icr.astype(np.float32), ics=ics.astype(np.float32),
    )
    _cache['tables'] = tabs
    return tabs


# ---------------------------------------------------------------- NEFF 1
def build_neff1():
    nc = bacc.Bacc(None, target_bir_lowering=False, debug=False)
    q_d = nc.declare_dram_parameter('q', [L, D], MM_DT, isOutput=False)
    k_d = nc.declare_dram_parameter('k', [L, D], MM_DT, isOutput=False)
    tct_d = nc.declare_dram_parameter('tct', [D, 512], MM_DT, isOutput=False)
    tst_d = nc.declare_dram_parameter('tst', [D, 512], MM_DT, isOutput=False)
    ones_d = nc.declare_dram_parameter('ones', [128, 2], MM_DT, isOutput=False)
    z_d = nc.declare_dram_parameter('zout', [2, 512], F32, isOutput=True)

    LT, DT = L // 128, D // 128        # 8, 4

    with tile.TileContext(nc) as tc, ExitStack() as ctx:
        pool = ctx.enter_context(tc.tile_pool(name='sb', bufs=1))
        psum = ctx.enter_context(
            tc.tile_pool(name='ps', bufs=2, space=bass.MemorySpace.PSUM))

        q_sb = pool.tile([128, LT, D], MM_DT)
        k_sb = pool.tile([128, LT, D], MM_DT)
        tct_sb = pool.tile([128, DT, 512], MM_DT)
        tst_sb = pool.tile([128, DT, 512], MM_DT)
        ones_sb = pool.tile([128, 2], MM_DT)
        nc.sync.dma_start(ones_sb[:], ones_d[:, :])
        for i in range(LT):
            nc.sync.dma_start(q_sb[:, i, :], q_d[i * 128:(i + 1) * 128, :])
            nc.sync.dma_start(k_sb[:, i, :], k_d[i * 128:(i + 1) * 128, :])
        for i in range(DT):
            nc.sync.dma_start(tct_sb[:, i, :], tct_d[i * 128:(i + 1) * 128, :])
            nc.sync.dma_start(tst_sb[:, i, :], tst_d[i * 128:(i + 1) * 128, :])
        # fp32r tiles hold plain IEEE fp32 bits; view them as f32 for the
        # elementwise U-stage so no separate f32 copies of the tables ship.
        def tct_f(t, sl=slice(None)):
            return tct_sb[:, t, sl].bitcast(F32)

        def tst_f(t, sl=slice(None)):
            return tst_sb[:, t, sl].bitcast(F32)

        # N[d2, d1] = sum_l k[l,d2] q[l,d1]
        n_sb = pool.tile([128, DT, D], MM_DT)
        for t2 in range(DT):
            pn = psum.tile([128, D], F32)
            for lt in range(LT):
                nc.tensor.matmul(
                    pn[:],
                    k_sb[:, lt, t2 * 128:(t2 + 1) * 128],
                    q_sb[:, lt, :],
                    start=(lt == 0), stop=(lt == LT - 1))
            nc.vector.tensor_copy(n_sb[:, t2, :], pn[:])

        # T1[d1, p] = sum_d2 N[d2,d1] tct[d2,p];  T2 with tst
        t1_sb = pool.tile([128, DT, 512], F32)
        t2_sb = pool.tile([128, DT, 512], F32)
        for d1t in range(DT):
            pt = psum.tile([128, 512], F32, tag='pt')
            for t2 in range(DT):
                nc.tensor.matmul(
                    pt[:],
                    n_sb[:, t2, d1t * 128:(d1t + 1) * 128],
                    tct_sb[:, t2, :],
                    start=(t2 == 0), stop=(t2 == DT - 1))
            nc.vector.tensor_copy(t1_sb[:, d1t, :], pt[:])
            pt2 = psum.tile([128, 512], F32, tag='pt')
            for t2 in range(DT):
                nc.tensor.matmul(
                    pt2[:],
                    n_sb[:, t2, d1t * 128:(d1t + 1) * 128],
                    tst_sb[:, t2, :],
                    start=(t2 == 0), stop=(t2 == DT - 1))
            nc.vector.tensor_copy(t2_sb[:, d1t, :], pt2[:])

        # u1 = cos*T1 + sin*T2 ; u2 = cos*T2 - sin*T1 (cos/sin = tct/tst
        # f32 views). tst col0 holds the Nyquist pack c512, so fix col 0:
        #   u1[:,0] := T1[:,0]           (pure Zr[0] contribution)
        #   u2[:,0] := c512 * T2[:,0]    (sums to Zr[512])
        u1_sb = pool.tile([128, DT, 512], MM_DT)
        u2_sb = pool.tile([128, DT, 512], MM_DT)
        for t in range(DT):
            ta = pool.tile([128, 512], F32, tag='ta')
            tb = pool.tile([128, 512], F32, tag='tb')
            nc.vector.tensor_mul(ta[:], tct_f(t), t1_sb[:, t, :])
            nc.vector.tensor_mul(tb[:], tst_f(t), t2_sb[:, t, :])
            nc.vector.tensor_add(u1_sb[:, t, :], ta[:], tb[:])
            tc_ = pool.tile([128, 512], F32, tag='tc')
            td = pool.tile([128, 512], F32, tag='td')
            nc.vector.tensor_mul(tc_[:], tct_f(t), t2_sb[:, t, :])
            nc.vector.tensor_mul(td[:], tst_f(t), t1_sb[:, t, :])
            nc.vector.tensor_sub(u2_sb[:, t, :], tc_[:], td[:])
            # col-0 fixups (tiny [128,1] ops)
            nc.vector.tensor_copy(u1_sb[:, t, 0:1], t1_sb[:, t, 0:1])
            nc.vector.tensor_mul(
                u2_sb[:, t, 0:1], tst_f(t, slice(0, 1)), t2_sb[:, t, 0:1])

        # Zr = sum_d u1, Zi = sum_d u2: PE ones-matmul partition reduce
        z_sb = pool.tile([2, 2, 512], F32)
        for row, u_sb in ((0, u1_sb), (1, u2_sb)):
            pz = psum.tile([2, 512], F32, tag='pz')
            for t in range(DT):
                nc.tensor.matmul(
                    pz[:], ones_sb[:], u_sb[:, t, :],
                    start=(t == 0), stop=(t == DT - 1))
            nc.vector.tensor_copy(z_sb[:, row, :], pz[:])
        nc.sync.dma_start(z_d[0:1, :], z_sb[0:1, 0, :])
        nc.sync.dma_start(z_d[1:2, :], z_sb[0:1, 1, :])

    nc.finalize()
    return nc


# ---------------------------------------------------------------- NEFF 2
def build_neff2():
    nc = bacc.Bacc(None, target_bir_lowering=False, debug=False)
    v_d = nc.declare_dram_parameter('v', [L, D], MM_DT, isOutput=False)
    fc_d = nc.declare_dram_parameter('fc', [L, 512], MM_DT, isOutput=False)
    fs_d = nc.declare_dram_parameter('fs', [L, 512], MM_DT, isOutput=False)
    icr_d = nc.declare_dram_parameter('icr', [512, L], MM_DT, isOutput=False)
    ics_d = nc.declare_dram_parameter('ics', [512, L], MM_DT, isOutput=False)
    a_d = nc.declare_dram_parameter('va', [512, 1], F32, isOutput=False)
    b_d = nc.declare_dram_parameter('vb', [512, 1], F32, isOutput=False)
    c_d = nc.declare_dram_parameter('vc', [512, 1], F32, isOutput=False)
    d_d = nc.declare_dram_parameter('vd', [512, 1], F32, isOutput=False)
    o_d = nc.declare_dram_parameter('out', [L, D], F32, isOutput=True)

    LT, PT = L // 128, 512 // 128      # 8, 4

    with tile.TileContext(nc) as tc, ExitStack() as ctx:
        pool = ctx.enter_context(tc.tile_pool(name='sb', bufs=1))
        outp = ctx.enter_context(tc.tile_pool(name='op', bufs=3))
        psum = ctx.enter_context(
            tc.tile_pool(name='ps', bufs=2, space=bass.MemorySpace.PSUM))
        psum_o = ctx.enter_context(
            tc.tile_pool(name='pso', bufs=2, space=bass.MemorySpace.PSUM))

        v_sb = pool.tile([128, LT, D], MM_DT)
        fc_sb = pool.tile([128, LT, 512], MM_DT)
        fs_sb = pool.tile([128, LT, 512], MM_DT)
        icr_sb = pool.tile([128, PT, L], MM_DT)
        ics_sb = pool.tile([128, PT, L], MM_DT)
        a_sb = pool.tile([128, PT, 1], F32)
        b_sb = pool.tile([128, PT, 1], F32)
        c_sb = pool.tile([128, PT, 1], F32)
        d_sb = pool.tile([128, PT, 1], F32)
        for i in range(LT):
            nc.sync.dma_start(v_sb[:, i, :], v_d[i * 128:(i + 1) * 128, :])
            nc.sync.dma_start(fc_sb[:, i, :], fc_d[i * 128:(i + 1) * 128, :])
            nc.sync.dma_start(fs_sb[:, i, :], fs_d[i * 128:(i + 1) * 128, :])
        for i in range(PT):
            nc.sync.dma_start(icr_sb[:, i, :], icr_d[i * 128:(i + 1) * 128, :])
            nc.sync.dma_start(ics_sb[:, i, :], ics_d[i * 128:(i + 1) * 128, :])
            nc.sync.dma_start(a_sb[:, i, :], a_d[i * 128:(i + 1) * 128, :])
            nc.sync.dma_start(b_sb[:, i, :], b_d[i * 128:(i + 1) * 128, :])
            nc.sync.dma_start(c_sb[:, i, :], c_d[i * 128:(i + 1) * 128, :])
            nc.sync.dma_start(d_sb[:, i, :], d_d[i * 128:(i + 1) * 128, :])

        # forward DFT along l: Vr[p,d], Vi[p,d]
        hr_sb = pool.tile([128, PT, D], MM_DT)
        hi_sb = pool.tile([128, PT, D], MM_DT)
        for pt in range(PT):
            pvr = psum.tile([128, D], F32, tag='pv')
            for lt in range(LT):
                nc.tensor.matmul(
                    pvr[:],
                    fc_sb[:, lt, pt * 128:(pt + 1) * 128],
                    v_sb[:, lt, :],
                    start=(lt == 0), stop=(lt == LT - 1))
            pvi = psum.tile([128, D], F32, tag='pv')
            for lt in range(LT):
                nc.tensor.matmul(
                    pvi[:],
                    fs_sb[:, lt, pt * 128:(pt + 1) * 128],
                    v_sb[:, lt, :],
                    start=(lt == 0), stop=(lt == LT - 1))
            # Hr = a*Vr + b*Vi ; Hi = c*Vi + d*Vr  (scalar = per-partition)
            tmp = pool.tile([128, D], F32, tag='htmp')
            nc.vector.tensor_scalar_mul(tmp[:], pvi[:], b_sb[:, pt, :])
            nc.vector.scalar_tensor_tensor(
                hr_sb[:, pt, :], pvr[:], a_sb[:, pt, :], tmp[:],
                mybir.AluOpType.mult, mybir.AluOpType.add)
            tmp2 = pool.tile([128, D], F32, tag='htmp2')
            nc.vector.tensor_scalar_mul(tmp2[:], pvr[:], d_sb[:, pt, :])
            nc.vector.scalar_tensor_tensor(
                hi_sb[:, pt, :], pvi[:], c_sb[:, pt, :], tmp2[:],
                mybir.AluOpType.mult, mybir.AluOpType.add)

        # inverse DFT: out[l,d] = sum_p icr[p,l] Hr[p,d] + ics[p,l] Hi[p,d]
        for lt in range(LT):
            po = psum_o.tile([128, D], F32)
            for pt in range(PT):
                nc.tensor.matmul(
                    po[:],
                    icr_sb[:, pt, lt * 128:(lt + 1) * 128],
                    hr_sb[:, pt, :],
                    start=(pt == 0), stop=False)
                nc.tensor.matmul(
                    po[:],
                    ics_sb[:, pt, lt * 128:(lt + 1) * 128],
                    hi_sb[:, pt, :],
                    start=False, stop=(pt == PT - 1))
            o_sb = outp.tile([128, D], F32)
            nc.vector.tensor_copy(o_sb[:], po[:])
            nc.sync.dma_start(o_d[lt * 128:(lt + 1) * 128, :], o_sb[:])

    nc.finalize()
    return nc


# ---------------------------------------------------------------- driver
def _get_graphs():
    if 'nc1' not in _cache:
        _cache['nc1'] = build_neff1()
        _cache['nc2'] = build_neff2()
    return _cache['nc1'], _cache['nc2']


def kernel(queries, keys, values, _trace=False):
    tabs = _tables()
    nc1, nc2 = _get_graphs()
    q = np.ascontiguousarray(np.asarray(queries, np.float32))
    k = np.ascontiguousarray(np.asarray(keys, np.float32))
    v = np.ascontiguousarray(np.asarray(values, np.float32))

    ones128 = np.ones((128, 2), np.float32)
    in1 = [{'q': q[b], 'k': k[b],
            'tct': tabs['tct'], 'tst': tabs['tst'], 'ones': ones128}
           for b in range(B)]
    r1 = run_bass_kernel_spmd(nc1, in1, core_ids=CORE_IDS, trace=_trace)
    z = np.stack([r1.results[b]['zout'] for b in range(B)])   # [B, 2, 512]

    # unpack: Zr[0..511] = z[:,0,:]; Zr[512] = z[:,1,0]; Zi[1..511] = z[:,1,1:]
    Zr = np.concatenate([z[:, 0, :], z[:, 1, 0:1]], axis=1)   # [B, 513]
    Zi = np.concatenate(
        [np.zeros((B, 1)), z[:, 1, 1:], np.zeros((B, 1))], axis=1)
    mean_value = Zr @ tabs['C2'] + Zi @ tabs['S2']            # [B, T]
    ind = np.argsort(-mean_value, axis=-1, kind='stable')[:, :K]
    val = np.take_along_axis(mean_value, ind, axis=-1)
    e = np.exp(val - val.max(-1, keepdims=True))
    w = e / e.sum(-1, keepdims=True)                          # [B, K]
    shifts = ind[0]                                           # [K]

    f = np.arange(NF)
    ang = 2 * np.pi * np.outer(f, shifts) / L                 # [513, K]
    cosm, sinm = np.cos(ang), np.sin(ang)
    cr = w @ cosm.T                                           # [B, 513]
    ci = -(w @ sinm.T)
    a_v = cr[:, :512].copy()
    b_v = ci[:, :512].copy(); b_v[:, 0] = 0.0
    c_v = cr[:, :512].copy(); c_v[:, 0] = cr[:, 512]
    d_v = -ci[:, :512].copy(); d_v[:, 0] = 0.0

    in2 = [{'v': v[b],
            'fc': tabs['fc'], 'fs': tabs['fs'],
            'icr': tabs['icr'], 'ics': tabs['ics'],
            'va': np.ascontiguousarray(a_v[b].reshape(512, 1), np.float32),
            'vb': np.ascontiguousarray(b_v[b].reshape(512, 1), np.float32),
            'vc': np.ascontiguousarray(c_v[b].reshape(512, 1), np.float32),
            'vd': np.ascontiguousarray(d_v[b].reshape(512, 1), np.float32)}
           for b in range(B)]
    r2 = run_bass_kernel_spmd(nc2, in2, core_ids=CORE_IDS, trace=_trace)
    out = np.stack([r2.results[b]['out'] for b in range(B)])  # [B, L, D]

    kernel._last_exec_ns = (
        (r1.exec_time_ns or 0) + (r2.exec_time_ns or 0)
        if (r1.exec_time_ns or r2.exec_time_ns) else None)
    kernel._last_results = (r1, r2)
    return out.astype(np.float32)


# revision 16
# speedup vs baseline: 2.4208x; 1.2194x over previous
"""AutoCorrelation (factor=3) Trainium2 kernel, 8-core batch-parallel.

Math: the reference's corr tensor [B,L,2047] is only ever used through its
mean over L. That mean collapses to quadratic forms of the Gram matrix
M_b = q_b^T k_b (one [512,512] matmul per batch):
    Zbar[f] = c_f^T M c_f + s_f^T M s_f  + i (c_f^T M s_f - s_f^T M c_f)
mean_value = irfft(Zbar/L, 2047) is a tiny [513]->[2047] cos/sin matmul
(done on host), and the final roll-sum is a circular correlation done
spectrally with constant DFT matrices (all dense matmuls on device).

The f=512 Nyquist column is packed into the (always-zero) sin slot f=0 on
both device stages so every tensor is exactly 512 wide (4x128 tiles, all
even sizes -> fp32r-legal). NEFF1's U-stage uses four independent host
tables so the packed slots unpack cleanly:
    u1 = uc1*T1 + us1*T2   (col0: 1*T1[:,0])         -> Zr[0..511]
    u2 = uc2*T2 - us1*T1   (col0: c512*T2[:,0])      -> [Zr[512], Zi[1..511]]
where the T-stage sin table has col0 := cos(pi*d) so T2[:,0] = M c512.

NEFF1 (per core, batch b): N = k^T q; T1 = N^T tcT; T2 = N^T tsT;
    u1/u2 elementwise; Zr/Zi = partition-sum (gpsimd) -> zout [2,512]
Host: mean_value -> top-20 -> softmax weights w[b], batch-0 shifts ->
    per-batch spectral coefficient vectors a,b,c,d [512].
NEFF2 (per core): Vr/Vi = DFT_L(v) (Nyquist packed into Vi row 0);
    Hr = a*Vr + b*Vi; Hi = c*Vi + d*Vr; out = ICr^T Hr + ICs^T Hi.
"""
import math
import numpy as np

from contextlib import ExitStack
from concourse import bass, mybir, tile, bacc
from concourse.bass_utils import run_bass_kernel_spmd

B, L, D = 8, 1024, 512
NF = L // 2 + 1      # 513
T = 2 * L - 1        # 2047
K = int(3 * math.log(float(L)))  # 20
F32 = mybir.dt.float32

# matmul compute dtype: float32 (safe) or float32r (full-rate fp32 path)
MM_DT = mybir.dt.float32r

NCORES = 8
CORE_IDS = list(range(NCORES))

_cache = {}


# ---------------------------------------------------------------- tables
def _tables():
    if 'tables' in _cache:
        return _cache['tables']
    d = np.arange(D)
    l_idx = np.arange(L)
    p = np.arange(512)
    f = np.arange(NF)

    ang1 = 2 * np.pi * np.outer(d, p) / L              # [512, 512]
    cosd = np.cos(ang1)
    sind = np.sin(ang1)
    c512 = np.cos(np.pi * d)                           # (-1)^d

    tct = cosd.copy()                                  # col0 = 1
    tst = sind.copy(); tst[:, 0] = c512                # T-stage pack
    uc1 = cosd.copy()
    uc2 = cosd.copy(); uc2[:, 0] = c512
    us1 = sind.copy()                                  # col0 = 0

    ang2 = 2 * np.pi * np.outer(f, np.arange(T)) / T   # [513, 2047]
    alpha = np.full(NF, 2.0); alpha[0] = 1.0
    C2 = alpha[:, None] * np.cos(ang2) / (T * L)
    S2 = -2.0 * np.sin(ang2) / (T * L); S2[0] = 0.0

    ang = 2 * np.pi * np.outer(l_idx, p) / L           # [1024, 512]
    fc = np.cos(ang)
    fs = -np.sin(ang); fs[:, 0] = (-1.0) ** l_idx      # col0 := Nyquist row
    alp = np.full(512, 2.0); alp[0] = 1.0
    icr = (alp[:, None] * np.cos(ang.T)) / L           # [512, 1024]
    ics = (-2.0 * np.sin(ang.T)) / L
    ics[0, :] = ((-1.0) ** l_idx) / L

    tabs = dict(
        tct=tct.astype(np.float32), tst=tst.astype(np.float32),
        uc1=uc1.astype(np.float32), uc2=uc2.astype(np.float32),
        us1=us1.astype(np.float32),
        C2=C2, S2=S2,
        fc=fc.astype(np.float32), fs=fs.astype(np.float32),
        icr=icr.astype(np.float32), ics=ics.astype(np.float32),
    )
    _cache['tables'] = tabs
    return tabs


# ---------------------------------------------------------------- NEFF 1
def build_neff1():
    nc = bacc.Bacc(None, target_bir_lowering=False, debug=False)
    q_d = nc.declare_dram_parameter('q', [L, D], MM_DT, isOutput=False)
    k_d = nc.declare_dram_parameter('k', [L, D], MM_DT, isOutput=False)
    tct_d = nc.declare_dram_parameter('tct', [D, 512], MM_DT, isOutput=False)
    tst_d = nc.declare_dram_parameter('tst', [D, 512], MM_DT, isOutput=False)
    ones_d = nc.declare_dram_parameter('ones', [128, 2], MM_DT, isOutput=False)
    z_d = nc.declare_dram_parameter('zout', [2, 512], F32, isOutput=True)

    LT, DT = L // 128, D // 128        # 8, 4

    with tile.TileContext(nc) as tc, ExitStack() as ctx:
        pool = ctx.enter_context(tc.tile_pool(name='sb', bufs=1))
        psum = ctx.enter_context(
            tc.tile_pool(name='ps', bufs=2, space=bass.MemorySpace.PSUM))

        q_sb = pool.tile([128, LT, D], MM_DT)
        k_sb = pool.tile([128, LT, D], MM_DT)
        tct_sb = pool.tile([128, DT, 512], MM_DT)
        tst_sb = pool.tile([128, DT, 512], MM_DT)
        ones_sb = pool.tile([128, 2], MM_DT)
        nc.sync.dma_start(ones_sb[:], ones_d[:, :])
        for i in range(LT):
            nc.sync.dma_start(q_sb[:, i, :], q_d[i * 128:(i + 1) * 128, :])
            nc.sync.dma_start(k_sb[:, i, :], k_d[i * 128:(i + 1) * 128, :])
        for i in range(DT):
            nc.sync.dma_start(tct_sb[:, i, :], tct_d[i * 128:(i + 1) * 128, :])
            nc.sync.dma_start(tst_sb[:, i, :], tst_d[i * 128:(i + 1) * 128, :])
        # fp32r tiles hold plain IEEE fp32 bits; view them as f32 for the
        # elementwise U-stage so no separate f32 copies of the tables ship.
        def tct_f(t, sl=slice(None)):
            return tct_sb[:, t, sl].bitcast(F32)

        def tst_f(t, sl=slice(None)):
            return tst_sb[:, t, sl].bitcast(F32)

        # N[d2, d1] = sum_l k[l,d2] q[l,d1]
        n_sb = pool.tile([128, DT, D], MM_DT)
        for t2 in range(DT):
            pn = psum.tile([128, D], F32)
            for lt in range(LT):
                nc.tensor.matmul(
                    pn[:],
                    k_sb[:, lt, t2 * 128:(t2 + 1) * 128],
                    q_sb[:, lt, :],
                    start=(lt == 0), stop=(lt == LT - 1))
            nc.vector.tensor_copy(n_sb[:, t2, :], pn[:])

        # T1[d1, p] = sum_d2 N[d2,d1] tct[d2,p];  T2 with tst
        t1_sb = pool.tile([128, DT, 512], F32)
        t2_sb = pool.tile([128, DT, 512], F32)
        for d1t in range(DT):
            pt = psum.tile([128, 512], F32, tag='pt')
            for t2 in range(DT):
                nc.tensor.matmul(
                    pt[:],
                    n_sb[:, t2, d1t * 128:(d1t + 1) * 128],
                    tct_sb[:, t2, :],
                    start=(t2 == 0), stop=(t2 == DT - 1))
            nc.vector.tensor_copy(t1_sb[:, d1t, :], pt[:])
            pt2 = psum.tile([128, 512], F32, tag='pt')
            for t2 in range(DT):
                nc.tensor.matmul(
                    pt2[:],
                    n_sb[:, t2, d1t * 128:(d1t + 1) * 128],
                    tst_sb[:, t2, :],
                    start=(t2 == 0), stop=(t2 == DT - 1))
            nc.vector.tensor_copy(t2_sb[:, d1t, :], pt2[:])

        # u1 = cos*T1 + sin*T2 ; u2 = cos*T2 - sin*T1 (cos/sin = tct/tst
        # f32 views). tst col0 holds the Nyquist pack c512, so fix col 0:
        #   u1[:,0] := T1[:,0]           (pure Zr[0] contribution)
        #   u2[:,0] := c512 * T2[:,0]    (sums to Zr[512])
        u1_sb = pool.tile([128, DT, 512], MM_DT)
        u2_sb = pool.tile([128, DT, 512], MM_DT)
        for t in range(DT):
            ta = pool.tile([128, 512], F32, tag='ta')
            tb = pool.tile([128, 512], F32, tag='tb')
            nc.vector.tensor_mul(ta[:], tct_f(t), t1_sb[:, t, :])
            nc.vector.tensor_mul(tb[:], tst_f(t), t2_sb[:, t, :])
            nc.vector.tensor_add(u1_sb[:, t, :], ta[:], tb[:])
            tc_ = pool.tile([128, 512], F32, tag='tc')
            td = pool.tile([128, 512], F32, tag='td')
            nc.vector.tensor_mul(tc_[:], tct_f(t), t2_sb[:, t, :])
            nc.vector.tensor_mul(td[:], tst_f(t), t1_sb[:, t, :])
            nc.vector.tensor_sub(u2_sb[:, t, :], tc_[:], td[:])
            # col-0 fixups (tiny [128,1] ops)
            nc.vector.tensor_copy(u1_sb[:, t, 0:1], t1_sb[:, t, 0:1])
            nc.vector.tensor_mul(
                u2_sb[:, t, 0:1], tst_f(t, slice(0, 1)), t2_sb[:, t, 0:1])

        # Zr = sum_d u1, Zi = sum_d u2: PE ones-matmul partition reduce
        z_sb = pool.tile([2, 2, 512], F32)
        for row, u_sb in ((0, u1_sb), (1, u2_sb)):
            pz = psum.tile([2, 512], F32, tag='pz')
            for t in range(DT):
                nc.tensor.matmul(
                    pz[:], ones_sb[:], u_sb[:, t, :],
                    start=(t == 0), stop=(t == DT - 1))
            nc.vector.tensor_copy(z_sb[:, row, :], pz[:])
        nc.sync.dma_start(z_d[0:1, :], z_sb[0:1, 0, :])
        nc.sync.dma_start(z_d[1:2, :], z_sb[0:1, 1, :])

    nc.finalize()
    return nc


# ---------------------------------------------------------------- NEFF 2
def build_neff2():
    """out[l,d] = sum_m At[m,l] v[m,d] with At[m,l] = coef[(m-l) mod L]:
    the weighted roll-sum is a circulant matmul (one [1024,1024]@[1024,512]
    per batch), At built on host from the 20 softmax weights."""
    nc = bacc.Bacc(None, target_bir_lowering=False, debug=False)
    v_d = nc.declare_dram_parameter('v', [L, D], MM_DT, isOutput=False)
    at_d = nc.declare_dram_parameter('at', [L, L], MM_DT, isOutput=False)
    o_d = nc.declare_dram_parameter('out', [L, D], F32, isOutput=True)

    LT = L // 128                      # 8

    with tile.TileContext(nc) as tc, ExitStack() as ctx:
        pool = ctx.enter_context(tc.tile_pool(name='sb', bufs=1))
        outp = ctx.enter_context(tc.tile_pool(name='op', bufs=3))
        psum_o = ctx.enter_context(
            tc.tile_pool(name='pso', bufs=2, space=bass.MemorySpace.PSUM))

        v_sb = pool.tile([128, LT, D], MM_DT)
        at_sb = pool.tile([128, LT, L], MM_DT)
        for i in range(LT):
            nc.sync.dma_start(v_sb[:, i, :], v_d[i * 128:(i + 1) * 128, :])
            nc.sync.dma_start(at_sb[:, i, :], at_d[i * 128:(i + 1) * 128, :])

        # out[l,d] = sum_m At[m,l] v[m,d]
        for lt in range(LT):
            po = psum_o.tile([128, D], F32)
            for mt in range(LT):
                nc.tensor.matmul(
                    po[:],
                    at_sb[:, mt, lt * 128:(lt + 1) * 128],
                    v_sb[:, mt, :],
                    start=(mt == 0), stop=(mt == LT - 1))
            o_sb = outp.tile([128, D], F32)
            nc.vector.tensor_copy(o_sb[:], po[:])
            nc.sync.dma_start(o_d[lt * 128:(lt + 1) * 128, :], o_sb[:])

    nc.finalize()
    return nc


# ---------------------------------------------------------------- driver
def _get_graphs():
    if 'nc1' not in _cache:
        _cache['nc1'] = build_neff1()
        _cache['nc2'] = build_neff2()
    return _cache['nc1'], _cache['nc2']


def kernel(queries, keys, values, _trace=False):
    tabs = _tables()
    nc1, nc2 = _get_graphs()
    q = np.ascontiguousarray(np.asarray(queries, np.float32))
    k = np.ascontiguousarray(np.asarray(keys, np.float32))
    v = np.ascontiguousarray(np.asarray(values, np.float32))

    ones128 = np.ones((128, 2), np.float32)
    in1 = [{'q': q[b], 'k': k[b],
            'tct': tabs['tct'], 'tst': tabs['tst'], 'ones': ones128}
           for b in range(B)]
    r1 = run_bass_kernel_spmd(nc1, in1, core_ids=CORE_IDS, trace=_trace)
    z = np.stack([r1.results[b]['zout'] for b in range(B)])   # [B, 2, 512]

    # unpack: Zr[0..511] = z[:,0,:]; Zr[512] = z[:,1,0]; Zi[1..511] = z[:,1,1:]
    Zr = np.concatenate([z[:, 0, :], z[:, 1, 0:1]], axis=1)   # [B, 513]
    Zi = np.concatenate(
        [np.zeros((B, 1)), z[:, 1, 1:], np.zeros((B, 1))], axis=1)
    mean_value = Zr @ tabs['C2'] + Zi @ tabs['S2']            # [B, T]
    ind = np.argsort(-mean_value, axis=-1, kind='stable')[:, :K]
    val = np.take_along_axis(mean_value, ind, axis=-1)
    e = np.exp(val - val.max(-1, keepdims=True))
    w = e / e.sum(-1, keepdims=True)                          # [B, K]
    shifts = ind[0]                                           # [K]

    # circulant build: coef[j] = sum_k w[b,k] [j == s_k mod L];
    # At[m,l] = coef[(m-l) mod L] via an as_strided view of 3x-tiled coef
    sh = shifts % L
    ats = []
    for b in range(B):
        coef = np.zeros(L, np.float32)
        np.add.at(coef, sh, w[b].astype(np.float32))
        coef3 = np.concatenate([coef, coef, coef])
        view = np.lib.stride_tricks.as_strided(
            coef3[L:], shape=(L, L), strides=(4, -4))
        ats.append(np.ascontiguousarray(view))

    in2 = [{'v': v[b], 'at': ats[b]} for b in range(B)]
    r2 = run_bass_kernel_spmd(nc2, in2, core_ids=CORE_IDS, trace=_trace)
    out = np.stack([r2.results[b]['out'] for b in range(B)])  # [B, L, D]

    kernel._last_exec_ns = (
        (r1.exec_time_ns or 0) + (r2.exec_time_ns or 0)
        if (r1.exec_time_ns or r2.exec_time_ns) else None)
    kernel._last_results = (r1, r2)
    return out.astype(np.float32)


# revision 20
# speedup vs baseline: 2.6462x; 1.0931x over previous
"""AutoCorrelation (factor=3) Trainium2 kernel, 8-core batch-parallel.

Math: the reference's corr tensor [B,L,2047] is only ever used through its
mean over L. That mean collapses to quadratic forms of the Gram matrix
M_b = q_b^T k_b (one [512,512] matmul per batch):
    Zbar[f] = c_f^T M c_f + s_f^T M s_f  + i (c_f^T M s_f - s_f^T M c_f)
mean_value = irfft(Zbar/L, 2047) is a tiny [513]->[2047] cos/sin matmul
(done on host), and the final roll-sum is a circular correlation done
spectrally with constant DFT matrices (all dense matmuls on device).

The f=512 Nyquist column is packed into the (always-zero) sin slot f=0 on
both device stages so every tensor is exactly 512 wide (4x128 tiles, all
even sizes -> fp32r-legal). NEFF1's U-stage uses four independent host
tables so the packed slots unpack cleanly:
    u1 = uc1*T1 + us1*T2   (col0: 1*T1[:,0])         -> Zr[0..511]
    u2 = uc2*T2 - us1*T1   (col0: c512*T2[:,0])      -> [Zr[512], Zi[1..511]]
where the T-stage sin table has col0 := cos(pi*d) so T2[:,0] = M c512.

NEFF1 (per core, batch b): N = k^T q; T1 = N^T tcT; T2 = N^T tsT;
    u1/u2 elementwise; Zr/Zi = partition-sum (gpsimd) -> zout [2,512]
Host: mean_value -> top-20 -> softmax weights w[b], batch-0 shifts ->
    per-batch spectral coefficient vectors a,b,c,d [512].
NEFF2 (per core): Vr/Vi = DFT_L(v) (Nyquist packed into Vi row 0);
    Hr = a*Vr + b*Vi; Hi = c*Vi + d*Vr; out = ICr^T Hr + ICs^T Hi.
"""
import math
import numpy as np

from contextlib import ExitStack
from concourse import bass, mybir, tile, bacc
from concourse.bass_utils import run_bass_kernel_spmd

B, L, D = 8, 1024, 512
NF = L // 2 + 1      # 513
T = 2 * L - 1        # 2047
K = int(3 * math.log(float(L)))  # 20
F32 = mybir.dt.float32

# matmul compute dtype: float32 (safe) or float32r (full-rate fp32 path)
MM_DT = mybir.dt.float32r

NCORES = 8
CORE_IDS = list(range(NCORES))

_cache = {}


# ---------------------------------------------------------------- tables
def _tables():
    """KER[j, t]: mean_value = G @ KER, where G[j] is the diagonal sum of
    N = k^T q at offset Delta = j - 512. Combines the d-axis DFT of G with
    the irfft-to-2047 of Zbar/L (both tiny, fused into one [1024, 2047]
    host matrix)."""
    if 'tables' in _cache:
        return _cache['tables']
    f = np.arange(NF)

    ang2 = 2 * np.pi * np.outer(f, np.arange(T)) / T   # [513, 2047]
    alpha = np.full(NF, 2.0); alpha[0] = 1.0
    C2 = alpha[:, None] * np.cos(ang2) / (T * L)
    S2 = -2.0 * np.sin(ang2) / (T * L); S2[0] = 0.0

    delta = np.arange(1024) - 512                      # [1024]
    angd = 2 * np.pi * np.outer(delta, f) / L          # [1024, 513]
    KER = np.cos(angd) @ C2 - np.sin(angd) @ S2        # [1024, 2047]

    tabs = dict(KER=KER)
    _cache['tables'] = tabs
    return tabs


# ---------------------------------------------------------------- NEFF 1
def build_neff1():
    """Zbar[f] = sum_Delta G[Delta] e^{-i 2pi f Delta / L} where
    G[Delta] = sum of the Delta-th diagonal of N = k^T q (Delta in
    [-511, 511]). Compute N on the PE, bounce it through a zero-padded
    DRAM buffer laid out [512 rows x 1536 cols] (zeros | N | zeros), and
    re-read with a skewed AP (partition stride = 1537 elements) so row p
    lands shifted by p: column sums of the two skewed views give the
    positive/negative diagonal sums directly. The DFT of G happens on
    the host (1024x2047 matmul, trivial)."""
    nc = bacc.Bacc(None, target_bir_lowering=False, debug=False)
    q_d = nc.declare_dram_parameter('q', [L, D], MM_DT, isOutput=False)
    k_d = nc.declare_dram_parameter('k', [L, D], MM_DT, isOutput=False)
    ones_d = nc.declare_dram_parameter('ones', [128, 2], MM_DT, isOutput=False)
    z_d = nc.declare_dram_parameter('zout', [2, 512], F32, isOutput=True)

    LT, DT = L // 128, D // 128        # 8, 4
    ROWW = 3 * 512                     # padded row width in the bounce buf
    SKEW = ROWW + 1

    with tile.TileContext(nc) as tc, ExitStack() as ctx:
        pool = ctx.enter_context(tc.tile_pool(name='sb', bufs=1))
        skp = ctx.enter_context(tc.tile_pool(name='sk', bufs=4))
        psum = ctx.enter_context(
            tc.tile_pool(name='ps', bufs=2, space=bass.MemorySpace.PSUM))
        psz = ctx.enter_context(
            tc.tile_pool(name='psz', bufs=2, space=bass.MemorySpace.PSUM))
        dram = ctx.enter_context(tc.tile_pool(name='dr', bufs=1, space='DRAM'))

        # flat bounce buffer; extra tail so the [128,1537] windows exist
        n2f = dram.tile([D * ROWW + 2048], MM_DT)

        def rows(t, w=ROWW):
            # [128, w]-strided view of row block t of the bounce buffer
            return n2f[t * 128 * w: (t + 1) * 128 * w].rearrange(
                '(p c) -> p c', c=w)

        def skew(t, plus):
            start = t * 128 * SKEW + (512 if plus else 0)
            return n2f[start: start + 128 * SKEW].rearrange(
                '(p c) -> p c', c=SKEW)[:, 0:512]

        q_sb = pool.tile([128, LT, D], MM_DT)
        k_sb = pool.tile([128, LT, D], MM_DT)
        ones_sb = pool.tile([128, 2], MM_DT)
        zero_f = pool.tile([128, 512], F32)
        zero_sb = pool.tile([128, 512], MM_DT)
        nc.sync.dma_start(ones_sb[:], ones_d[:, :])
        nc.vector.memset(zero_f[:], 0.0)
        nc.vector.tensor_copy(zero_sb[:], zero_f[:])
        for i in range(LT):
            nc.sync.dma_start(q_sb[:, i, :], q_d[i * 128:(i + 1) * 128, :])
            nc.sync.dma_start(k_sb[:, i, :], k_d[i * 128:(i + 1) * 128, :])
        # zero the pad columns (left 512, right 512 of each row block)
        for t in range(DT):
            nc.sync.dma_start(rows(t)[:, 0:512], zero_sb[:])
            nc.sync.dma_start(rows(t)[:, 1024:1536], zero_sb[:])

        # N[d2, d1] = sum_l k[l,d2] q[l,d1]; bounce rows to DRAM;
        # skew-read both diagonal halves; column-reduce via ones-matmul.
        pzp = psz.tile([2, 512], F32, tag='pzp')
        pzm = psz.tile([2, 512], F32, tag='pzm')
        for t2 in range(DT):
            pn = psum.tile([128, D], F32)
            for lt in range(LT):
                nc.tensor.matmul(
                    pn[:],
                    k_sb[:, lt, t2 * 128:(t2 + 1) * 128],
                    q_sb[:, lt, :],
                    start=(lt == 0), stop=(lt == LT - 1))
            n_t = pool.tile([128, 512], MM_DT, tag='nt')
            nc.vector.tensor_copy(n_t[:], pn[:])
            nc.sync.dma_start(rows(t2)[:, 512:1024], n_t[:])
            xp = skp.tile([128, 512], MM_DT, tag='xp')
            xm = skp.tile([128, 512], MM_DT, tag='xm')
            nc.sync.dma_start(xp[:], skew(t2, True))
            nc.sync.dma_start(xm[:], skew(t2, False))
            nc.tensor.matmul(pzp[:], ones_sb[:], xp[:],
                             start=(t2 == 0), stop=(t2 == DT - 1))
            nc.tensor.matmul(pzm[:], ones_sb[:], xm[:],
                             start=(t2 == 0), stop=(t2 == DT - 1))

        zp_sb = pool.tile([2, 512], F32, tag='zp')
        zm_sb = pool.tile([2, 512], F32, tag='zm')
        nc.vector.tensor_copy(zp_sb[:], pzp[:])
        nc.vector.tensor_copy(zm_sb[:], pzm[:])
        nc.sync.dma_start(z_d[0:1, :], zp_sb[0:1, :])
        nc.sync.dma_start(z_d[1:2, :], zm_sb[0:1, :])

    nc.finalize()
    return nc


# ---------------------------------------------------------------- NEFF 2
def build_neff2():
    """out[l,d] = sum_m At[m,l] v[m,d] with At[m,l] = coef[(m-l) mod L]:
    the weighted roll-sum is a circulant matmul (one [1024,1024]@[1024,512]
    per batch), At built on host from the 20 softmax weights."""
    nc = bacc.Bacc(None, target_bir_lowering=False, debug=False)
    v_d = nc.declare_dram_parameter('v', [L, D], MM_DT, isOutput=False)
    at_d = nc.declare_dram_parameter('at', [L, L], MM_DT, isOutput=False)
    o_d = nc.declare_dram_parameter('out', [L, D], F32, isOutput=True)

    LT = L // 128                      # 8

    with tile.TileContext(nc) as tc, ExitStack() as ctx:
        pool = ctx.enter_context(tc.tile_pool(name='sb', bufs=1))
        outp = ctx.enter_context(tc.tile_pool(name='op', bufs=3))
        psum_o = ctx.enter_context(
            tc.tile_pool(name='pso', bufs=2, space=bass.MemorySpace.PSUM))

        v_sb = pool.tile([128, LT, D], MM_DT)
        at_sb = pool.tile([128, LT, L], MM_DT)
        for i in range(LT):
            nc.sync.dma_start(v_sb[:, i, :], v_d[i * 128:(i + 1) * 128, :])
            nc.sync.dma_start(at_sb[:, i, :], at_d[i * 128:(i + 1) * 128, :])

        # out[l,d] = sum_m At[m,l] v[m,d]
        for lt in range(LT):
            po = psum_o.tile([128, D], F32)
            for mt in range(LT):
                nc.tensor.matmul(
                    po[:],
                    at_sb[:, mt, lt * 128:(lt + 1) * 128],
                    v_sb[:, mt, :],
                    start=(mt == 0), stop=(mt == LT - 1))
            o_sb = outp.tile([128, D], F32)
            nc.vector.tensor_copy(o_sb[:], po[:])
            nc.sync.dma_start(o_d[lt * 128:(lt + 1) * 128, :], o_sb[:])

    nc.finalize()
    return nc


# ---------------------------------------------------------------- driver
def _get_graphs():
    if 'nc1' not in _cache:
        _cache['nc1'] = build_neff1()
        _cache['nc2'] = build_neff2()
    return _cache['nc1'], _cache['nc2']


def kernel(queries, keys, values, _trace=False):
    tabs = _tables()
    nc1, nc2 = _get_graphs()
    q = np.ascontiguousarray(np.asarray(queries, np.float32))
    k = np.ascontiguousarray(np.asarray(keys, np.float32))
    v = np.ascontiguousarray(np.asarray(values, np.float32))

    ones128 = np.ones((128, 2), np.float32)
    in1 = [{'q': q[b], 'k': k[b], 'ones': ones128} for b in range(B)]
    r1 = run_bass_kernel_spmd(nc1, in1, core_ids=CORE_IDS, trace=_trace)
    z = np.stack([r1.results[b]['zout'] for b in range(B)])   # [B, 2, 512]

    # g[j] = diagonal sum of N at Delta = j - 512: row1 = negative half
    g = np.concatenate([z[:, 1, :], z[:, 0, :]], axis=1)      # [B, 1024]
    mean_value = g @ tabs['KER']                              # [B, T]
    ind = np.argsort(-mean_value, axis=-1, kind='stable')[:, :K]
    val = np.take_along_axis(mean_value, ind, axis=-1)
    e = np.exp(val - val.max(-1, keepdims=True))
    w = e / e.sum(-1, keepdims=True)                          # [B, K]
    shifts = ind[0]                                           # [K]

    # circulant build: coef[j] = sum_k w[b,k] [j == s_k mod L];
    # At[m,l] = coef[(m-l) mod L] via an as_strided view of 3x-tiled coef
    sh = shifts % L
    ats = []
    for b in range(B):
        coef = np.zeros(L, np.float32)
        np.add.at(coef, sh, w[b].astype(np.float32))
        coef3 = np.concatenate([coef, coef, coef])
        view = np.lib.stride_tricks.as_strided(
            coef3[L:], shape=(L, L), strides=(4, -4))
        ats.append(np.ascontiguousarray(view))

    in2 = [{'v': v[b], 'at': ats[b]} for b in range(B)]
    r2 = run_bass_kernel_spmd(nc2, in2, core_ids=CORE_IDS, trace=_trace)
    out = np.stack([r2.results[b]['out'] for b in range(B)])  # [B, L, D]

    kernel._last_exec_ns = (
        (r1.exec_time_ns or 0) + (r2.exec_time_ns or 0)
        if (r1.exec_time_ns or r2.exec_time_ns) else None)
    kernel._last_results = (r1, r2)
    return out.astype(np.float32)
